# revision 22
# baseline (speedup 1.0000x reference)
"""Direct N-body gravitational acceleration on 8 Trainium2 NeuronCores.

Sharding: target-particle axis j split across the 8 cores (1024 targets
per core); every core holds the full (N,3) source positions.

Math (reference):
    z[i,j]   = |y_i - y_j|^2 + eps
    icd[i,j] = z^{-3/2}
    acc[j]   = G*m_j * (sum_i icd[i,j]*y_i  -  y_j * sum_i icd[i,j])

Per-core device pipeline (everything O(N^2) stays on-chip):
    mm1 (PE): z[i,j] = a_i . b_j with a_i=(y_i, d2_i, 1),
        b_j=(-2*y_j, 1, d2_j+eps).  To avoid the 4x-cost fp32 matmul
        path each fp32 feature is 3-way fp16 split and the product
        expanded into 6 cross terms -> one K=30 fp16 matmul with
        fp32-grade accuracy.
    ACT: t = Abs_reciprocal_sqrt(z) = z^{-1/2}  (HW-measured 4.4e-5
        max rel err), one pass.
    DVE (custom fused op): icd/16 = min(t, 100)^3 / 16 written as fp16.
        The clamp bounds fp16 at 62500 (z rounding can dip below eps on
        the diagonal; true off-diagonal pairs sit well above the clamp).
    mm2 (PE, fp16, K=128 per i-tile, PSUM-accumulated over 64 i-tiles):
        S[c,j] += sum_i yp[i,c]*icd[i,j], with yp = [y | 1] split into
        fp16 hi+lo halves packed as one [128, 8] weight (the y_i - y_j
        cancellation must survive quantization) -> one matmul per
        (i-tile, j-half).
Host does the O(N) prep (feature splits) and the O(N) affine combine
    acc[j] = 16*G*m_j*(S[0:3,j] - y_j*S[3,j]) with S = S_hi + S_lo.
"""

import numpy as np

N = 8192
NCORES = 8
JL = N // NCORES  # 1024 local targets per core
P = 128
ITILES = N // P  # 64
EPS = np.float32(0.01 * 0.01)
ICD_SCALE = 16.0  # icd stored as icd/16 in fp16
T_CLAMP = 100.0  # = EPS**-0.5; min(t, clamp)^3/16 = 62500 < fp16 max
KF = 30  # feature rows after 3-way fp16 split (6 kept cross-product pairs)

_cache: dict = {}
LAST_RUN = None  # BassKernelResults of the most recent launch (for test.py)


def _register_cube_op():
    """Register a fused clamp+cube+scale custom DVE op (the documented
    dve_ops extension point, applied at runtime since the repo is read-only):
    out = min(in0, s0)^3 * s1."""
    import concourse.dve_ops as dve_ops
    from concourse.dve_spec import Spec, Src0, C0, C1, lower, minn, sq
    from concourse.dve_uop import DveOpSpec

    name = "CUBE_CLAMP_SCALE_NB"
    for op in dve_ops.OPS:
        if op.name == name:
            return op

    m = minn(Src0, C0)
    spec = Spec(
        body=sq(m) * m * C1,
        reference=lambda in0, in1, s0, s1, imm2: (
            np.minimum(in0.astype(np.float32), s0) ** 3 * s1
        ),
    )
    row = dve_ops._CUSTOM_DVE_ROW_BASE + len(dve_ops.OPS)
    shas = {}
    for ver in ("v3", "v4"):
        try:
            uops = lower(spec, ver=ver)
        except Exception:
            continue
        shas[ver] = DveOpSpec(name=name, opcode=row, uops=uops, rd1_en=False).sha(ver)
    op = dve_ops.DveOp(name, spec, subdim=False, uops_sha=shas)
    dve_ops.OPS.append(op)
    dve_ops.CUSTOM_DVE_SPECS[name] = spec
    dve_ops._SUB_OPCODE_FOR_NAME[name] = row
    return op


def _enable_ldw_opt():
    """The stock compile driver passes --enable-ldw-opt=false, which forces a
    serializing LDWEIGHTS before every matmul.  Enable the walrus LDW
    optimization (dedup/background-buffer loads) for this kernel's compiles."""
    import concourse.bass_utils as bu

    if getattr(bu, "_ldw_opt_patched", False):
        return
    orig = bu.bir_verify_and_optimise

    def patched(tmpdir, inp="bir.json", outp="file.neff", arch=None, *, dve_root=None):
        import unittest.mock as mock

        real_run = bu.run_command

        def run_patched(argv, **kw):
            argv = [
                a.replace("--enable-ldw-opt=false", "--enable-ldw-opt=true")
                for a in argv
            ]
            return real_run(argv, **kw)

        with mock.patch.object(bu, "run_command", run_patched):
            return orig(tmpdir, inp, outp, arch, dve_root=dve_root)

    bu.bir_verify_and_optimise = patched
    bu._ldw_opt_patched = True


def _build():
    import concourse.bacc as bacc
    import concourse.mybir as mybir
    import concourse.tile as tile

    f32 = mybir.dt.float32
    f16 = mybir.dt.float16
    AF = mybir.ActivationFunctionType

    cube_op = _register_cube_op()

    nc = bacc.Bacc("TRN2", target_bir_lowering=False, debug=False)
    # aTP packs the K=30 feature panels of two consecutive i-tiles into row
    # strips at partitions 0-29 and 32-61 so the two mm1 matmuls of a pair
    # run concurrently in different PE row groups (tile_position row tiling).
    aTP = nc.dram_tensor("aTP", [P, (N // 2)], f16, kind="ExternalInput")
    bTP = nc.dram_tensor("bTP", [P, JL], f16, kind="ExternalInput")
    ypc = nc.dram_tensor("ypc", [N, 8], f16, kind="ExternalInput")
    S = nc.dram_tensor("S", [8, JL], f32, kind="ExternalOutput")

    with tile.TileContext(nc) as tc:
        with (
            tc.tile_pool(name="const", bufs=1) as cpool,
            tc.tile_pool(name="tp", bufs=4) as tpool,
            tc.tile_pool(name="icdp", bufs=4) as icdpool,
            tc.tile_pool(name="ps1", bufs=3, space="PSUM") as ps1pool,
            tc.tile_pool(name="ps2", bufs=1, space="PSUM") as ps2pool,
            tc.tile_pool(name="outp", bufs=1) as opool,
        ):
            aTP_sb = cpool.tile([P, N // 2], f16)
            nc.sync.dma_start(aTP_sb[:], aTP[:])
            bTP_sb = cpool.tile([P, JL], f16)
            nc.sync.dma_start(bTP_sb[:], bTP[:])
            ypc_sb = cpool.tile([P, ITILES, 8], f16)
            nc.sync.dma_start(ypc_sb[:], ypc.rearrange("(t p) c -> p t c", p=P))

            ps2a = ps2pool.tile([8, 512], f32)
            ps2b = ps2pool.tile([8, 512], f32)

            # PE warm-up: ~10us of dense dependency-free matmuls on a zeroed
            # tile trips the HAM clock gate to 8/8 (2.4 GHz) before the main
            # loop; otherwise the whole kernel can run at the cold 1.2 GHz.
            # Output goes to the ps2 banks, which the first real accumulating
            # matmul resets via start=True.
            warm_in = cpool.tile([P, 512], f16)
            nc.vector.memset(warm_in[:], 0.0)
            for w in range(16):
                nc.tensor.matmul(
                    ps2a[:] if w % 2 == 0 else ps2b[:],
                    warm_in[:, 0:8],
                    warm_in[:],
                    start=True,
                    stop=True,
                )

            NG = ITILES // 2  # 32 groups of 2 i-tiles

            def emit_mm1_pair(g):
                # two concurrent row-tiled matmuls (strips at partitions 0/32)
                # compute z for i-tiles 2g and 2g+1
                tiles = []
                for r in (0, 1):
                    ps1 = ps1pool.tile([P, JL], f32, tag="ps1t")
                    lhs = aTP_sb[32 * r : 32 * r + KF, g * P : (g + 1) * P]
                    for jh in (0, 1):
                        nc.tensor.matmul(
                            ps1[:, jh * 512 : (jh + 1) * 512],
                            lhs,
                            bTP_sb[32 * r : 32 * r + KF, jh * 512 : (jh + 1) * 512],
                            start=True,
                            stop=True,
                            tile_position=(32 * r, 0),
                        )
                    tiles.append(ps1)
                return tiles

            # software pipeline: mm1 for group g+1 is queued before mm2 of
            # group g so the PE always has dependency-free work in flight
            cur = emit_mm1_pair(0)
            for g in range(NG):
                icds = []
                for r in (0, 1):
                    t = tpool.tile([P, JL], f32)
                    nc.scalar.activation(t[:], cur[r][:], AF.Abs_reciprocal_sqrt)
                    icd = icdpool.tile([P, JL], f16)
                    nc.vector._custom_dve(
                        cube_op, out=icd[:], in0=t[:], s0=T_CLAMP, s1=1.0 / ICD_SCALE
                    )
                    icds.append(icd)
                if g + 1 < NG:
                    cur = emit_mm1_pair(g + 1)
                for r in (0, 1):
                    it = 2 * g + r
                    first, last = it == 0, it == ITILES - 1
                    nc.tensor.matmul(
                        ps2a[:],
                        ypc_sb[:, it, :],
                        icds[r][:, 0:512],
                        start=first,
                        stop=last,
                    )
                    nc.tensor.matmul(
                        ps2b[:],
                        ypc_sb[:, it, :],
                        icds[r][:, 512:1024],
                        start=first,
                        stop=last,
                    )
            S_sb = opool.tile([8, JL], f32)
            nc.vector.tensor_copy(S_sb[:, 0:512], ps2a[:])
            nc.vector.tensor_copy(S_sb[:, 512:1024], ps2b[:])
            nc.sync.dma_start(S[:], S_sb[:])
    nc.compile()
    return nc


def _split16(x):
    hi = x.astype(np.float16)
    lo = (x - hi.astype(np.float32)).astype(np.float16)
    return hi, lo


def _split16_3(x):
    h = x.astype(np.float16)
    r = x - h.astype(np.float32)
    m = r.astype(np.float16)
    l = (r - m.astype(np.float32)).astype(np.float16)
    return h, m, l


def kernel(t, y, masses, G):
    global LAST_RUN
    from concourse.bass_utils import run_bass_kernel_spmd

    y = np.asarray(y, np.float32).reshape(N, 3)
    m = np.asarray(masses, np.float32).reshape(N)
    g = np.float32(np.asarray(G).reshape(()))

    d2 = (y * y).sum(1, dtype=np.float32)
    ones = np.ones(N, np.float32)
    a = np.stack([y[:, 0], y[:, 1], y[:, 2], d2, ones])  # [5, N] fp32
    b = np.stack([-2 * y[:, 0], -2 * y[:, 1], -2 * y[:, 2], ones, d2 + EPS])
    ah, am, al = _split16_3(a)
    bh, bm, bl = _split16_3(b)
    # (ah+am+al).(bh+bm+bl) expanded, keeping pairs whose product can reach
    # ~2^-22 of z: (h,h) (h,m) (m,h) (h,l) (l,h) (m,m); dropped terms < 2^-33.
    aT30 = np.concatenate([ah, ah, am, ah, al, am], axis=0)  # [30, N]
    bT30_full = np.concatenate([bh, bm, bh, bl, bh, bm], axis=0)  # [30, N]
    # pack pairs of i-tiles into row strips at partitions 0-29 / 32-61
    a3 = aT30.reshape(KF, ITILES // 2, 2, P)  # [30, 32, 2, 128]
    aTP = np.zeros((P, (ITILES // 2) * P), np.float16)
    aTP_v = aTP.reshape(P, ITILES // 2, P)
    aTP_v[0:KF] = a3[:, :, 0, :]
    aTP_v[32 : 32 + KF] = a3[:, :, 1, :]
    yp = np.concatenate([y, ones[:, None]], axis=1)  # [N, 4] fp32
    yph, ypl = _split16(yp)
    ypc = np.ascontiguousarray(np.concatenate([yph, ypl], axis=1))  # [N, 8]

    if "nc" not in _cache:
        _cache["nc"] = _build()
    nc = _cache["nc"]

    in_maps = []
    for c in range(NCORES):
        bT_loc = bT30_full[:, c * JL : (c + 1) * JL]
        bTP = np.zeros((P, JL), np.float16)
        bTP[0:KF] = bT_loc
        bTP[32 : 32 + KF] = bT_loc
        in_maps.append({"aTP": aTP, "bTP": bTP, "ypc": ypc})
    LAST_RUN = run_bass_kernel_spmd(nc, in_maps, core_ids=list(range(NCORES)))
    S8 = np.concatenate([r["S"] for r in LAST_RUN.results], axis=1)  # [8, N]
    S = S8[0:4] + S8[4:8]
    acc = (np.float32(ICD_SCALE) * g * m)[:, None] * (S[0:3].T - y * S[3][:, None])
    return acc.astype(np.float32)


# revision 25
# speedup vs baseline: 1.2469x; 1.2469x over previous
"""Direct N-body gravitational acceleration on 8 Trainium2 NeuronCores.

Sharding: target-particle axis j split across the 8 cores (1024 targets
per core); every core holds the full (N,3) source positions.

Math (reference):
    z[i,j]   = |y_i - y_j|^2 + eps
    icd[i,j] = z^{-3/2}
    acc[j]   = G*m_j * (sum_i icd[i,j]*y_i  -  y_j * sum_i icd[i,j])

Per-core device pipeline (everything O(N^2) stays on-chip):
    mm1 (PE): z[i,j] = a_i . b_j with a_i=(y_i, d2_i, 1),
        b_j=(-2*y_j, 1, d2_j+eps).  To avoid the 4x-cost fp32 matmul
        path each fp32 feature is 3-way fp16 split and the product
        expanded into 6 cross terms -> one K=30 fp16 matmul with
        fp32-grade accuracy.
    ACT: t = Abs_reciprocal_sqrt(z) = z^{-1/2}  (HW-measured 4.4e-5
        max rel err), one pass.
    DVE (custom fused op): icd/16 = min(t, 100)^3 / 16 written as fp16.
        The clamp bounds fp16 at 62500 (z rounding can dip below eps on
        the diagonal; true off-diagonal pairs sit well above the clamp).
    mm2 (PE, fp16, K=128 per i-tile, PSUM-accumulated over 64 i-tiles):
        S[c,j] += sum_i yp[i,c]*icd[i,j], with yp = [y | 1] split into
        fp16 hi+lo halves packed as one [128, 8] weight (the y_i - y_j
        cancellation must survive quantization) -> one matmul per
        (i-tile, j-half).
Host does the O(N) prep (feature splits) and the O(N) affine combine
    acc[j] = 16*G*m_j*(S[0:3,j] - y_j*S[3,j]) with S = S_hi + S_lo.
"""

import numpy as np

N = 8192
NCORES = 8
JL = N // NCORES  # 1024 local targets per core
P = 128
ITILES = N // P  # 64
EPS = np.float32(0.01 * 0.01)
ICD_SCALE = 16.0  # icd stored as icd/16 in fp16
T_CLAMP = 100.0  # = EPS**-0.5; min(t, clamp)^3/16 = 62500 < fp16 max
KF = 30  # feature rows after 3-way fp16 split (6 kept cross-product pairs)

_cache: dict = {}
LAST_RUN = None  # BassKernelResults of the most recent launch (for test.py)


def _register_cube_op():
    """Register a fused clamp+cube+scale custom DVE op (the documented
    dve_ops extension point, applied at runtime since the repo is read-only):
    out = min(in0, s0)^3 * s1."""
    import concourse.dve_ops as dve_ops
    from concourse.dve_spec import Spec, Src0, C0, C1, lower, minn, sq
    from concourse.dve_uop import DveOpSpec

    name = "CUBE_CLAMP_SCALE_NB"
    for op in dve_ops.OPS:
        if op.name == name:
            return op

    m = minn(Src0, C0)
    spec = Spec(
        body=sq(m) * m * C1,
        reference=lambda in0, in1, s0, s1, imm2: (
            np.minimum(in0.astype(np.float32), s0) ** 3 * s1
        ),
    )
    row = dve_ops._CUSTOM_DVE_ROW_BASE + len(dve_ops.OPS)
    shas = {}
    for ver in ("v3", "v4"):
        try:
            uops = lower(spec, ver=ver)
        except Exception:
            continue
        shas[ver] = DveOpSpec(name=name, opcode=row, uops=uops, rd1_en=False).sha(ver)
    op = dve_ops.DveOp(name, spec, subdim=False, uops_sha=shas)
    dve_ops.OPS.append(op)
    dve_ops.CUSTOM_DVE_SPECS[name] = spec
    dve_ops._SUB_OPCODE_FOR_NAME[name] = row
    return op


def _enable_ldw_opt():
    """The stock compile driver passes --enable-ldw-opt=false, which forces a
    serializing LDWEIGHTS before every matmul.  Enable the walrus LDW
    optimization (dedup/background-buffer loads) for this kernel's compiles."""
    import concourse.bass_utils as bu

    if getattr(bu, "_ldw_opt_patched", False):
        return
    orig = bu.bir_verify_and_optimise

    def patched(tmpdir, inp="bir.json", outp="file.neff", arch=None, *, dve_root=None):
        import unittest.mock as mock

        real_run = bu.run_command

        def run_patched(argv, **kw):
            argv = [
                a.replace("--enable-ldw-opt=false", "--enable-ldw-opt=true")
                for a in argv
            ]
            return real_run(argv, **kw)

        with mock.patch.object(bu, "run_command", run_patched):
            return orig(tmpdir, inp, outp, arch, dve_root=dve_root)

    bu.bir_verify_and_optimise = patched
    bu._ldw_opt_patched = True


def _build():
    import concourse.bacc as bacc
    import concourse.mybir as mybir
    import concourse.tile as tile

    f32 = mybir.dt.float32
    f16 = mybir.dt.float16
    AF = mybir.ActivationFunctionType

    cube_op = _register_cube_op()

    nc = bacc.Bacc("TRN2", target_bir_lowering=False, debug=False)
    # aTP packs the K=30 feature panels of two consecutive i-tiles into row
    # strips at partitions 0-29 and 32-61 so the two mm1 matmuls of a pair
    # run concurrently in different PE row groups (tile_position row tiling).
    aTP = nc.dram_tensor("aTP", [P, (N // 2)], f16, kind="ExternalInput")
    bTP = nc.dram_tensor("bTP", [P, JL], f16, kind="ExternalInput")
    ypc = nc.dram_tensor("ypc", [N, 8], f16, kind="ExternalInput")
    S = nc.dram_tensor("S", [8, JL], f32, kind="ExternalOutput")

    with tile.TileContext(nc) as tc:
        with (
            tc.tile_pool(name="const", bufs=1) as cpool,
            tc.tile_pool(name="tp", bufs=4) as tpool,
            tc.tile_pool(name="icdp", bufs=4) as icdpool,
            tc.tile_pool(name="ps1", bufs=2, space="PSUM") as ps1pool,
            tc.tile_pool(name="scr", bufs=1, space="PSUM") as scrpool,
            tc.tile_pool(name="ps2", bufs=1, space="PSUM") as ps2pool,
            tc.tile_pool(name="outp", bufs=1) as opool,
        ):
            aTP_sb = cpool.tile([P, N // 2], f16)
            nc.sync.dma_start(aTP_sb[:], aTP[:])
            bTP_sb = cpool.tile([P, JL], f16)
            nc.sync.dma_start(bTP_sb[:], bTP[:])
            ypc_sb = cpool.tile([P, ITILES, 8], f16)
            nc.sync.dma_start(ypc_sb[:], ypc.rearrange("(t p) c -> p t c", p=P))

            ps2a = ps2pool.tile([8, 512], f32)
            ps2b = ps2pool.tile([8, 512], f32)

            # PE warm-up: ~10us of dense dependency-free matmuls on a zeroed
            # tile trips the HAM clock gate to 8/8 (2.4 GHz) before the main
            # loop; otherwise the whole kernel can run at the cold 1.2 GHz.
            warm_in = cpool.tile([P, 512], f16)
            nc.vector.memset(warm_in[:], 0.0)
            scr = scrpool.tile([P, 512], f32)
            for w in range(16):
                nc.tensor.matmul(
                    scr[:], warm_in[:, 0:128], warm_in[:], start=True, stop=True
                )

            def emit_dummy_mm():
                # Dependency-free filler matmul.  The HAM clock gate
                # re-throttles the PE to 1.2 GHz whenever its busy fraction
                # drops; real work alone leaves the warm-clock PE ~70% busy,
                # which makes the throttle bi-stable at cold.  Padding the PE
                # queue keeps the duty cycle ~100% at 2.4 GHz, which is a
                # large net win.
                nc.tensor.matmul(
                    scr[:], warm_in[:, 0:128], warm_in[:], start=True, stop=True
                )

            NG = ITILES // 2  # 32 groups of 2 i-tiles

            def emit_mm1_pair(g):
                # two concurrent row-tiled matmuls (strips at partitions 0/32)
                # compute z for i-tiles 2g and 2g+1
                tiles = []
                for r in (0, 1):
                    ps1 = ps1pool.tile([P, JL], f32, tag="ps1t")
                    lhs = aTP_sb[32 * r : 32 * r + KF, g * P : (g + 1) * P]
                    for jh in (0, 1):
                        nc.tensor.matmul(
                            ps1[:, jh * 512 : (jh + 1) * 512],
                            lhs,
                            bTP_sb[32 * r : 32 * r + KF, jh * 512 : (jh + 1) * 512],
                            start=True,
                            stop=True,
                            tile_position=(32 * r, 0),
                        )
                    tiles.append(ps1)
                return tiles

            # software pipeline: mm1 for group g+1 is queued before mm2 of
            # group g so the PE always has dependency-free work in flight
            cur = emit_mm1_pair(0)
            for g in range(NG):
                icds = []
                for r in (0, 1):
                    t = tpool.tile([P, JL], f32)
                    nc.scalar.activation(t[:], cur[r][:], AF.Abs_reciprocal_sqrt)
                    icd = icdpool.tile([P, JL], f16)
                    nc.vector._custom_dve(
                        cube_op, out=icd[:], in0=t[:], s0=T_CLAMP, s1=1.0 / ICD_SCALE
                    )
                    icds.append(icd)
                if g + 1 < NG:
                    cur = emit_mm1_pair(g + 1)
                for r in (0, 1):
                    it = 2 * g + r
                    first, last = it == 0, it == ITILES - 1
                    nc.tensor.matmul(
                        ps2a[:],
                        ypc_sb[:, it, :],
                        icds[r][:, 0:512],
                        start=first,
                        stop=last,
                    )
                    nc.tensor.matmul(
                        ps2b[:],
                        ypc_sb[:, it, :],
                        icds[r][:, 512:1024],
                        start=first,
                        stop=last,
                    )
                    emit_dummy_mm()
                    emit_dummy_mm()
            S_sb = opool.tile([8, JL], f32)
            nc.vector.tensor_copy(S_sb[:, 0:512], ps2a[:])
            nc.vector.tensor_copy(S_sb[:, 512:1024], ps2b[:])
            nc.sync.dma_start(S[:], S_sb[:])
    nc.compile()
    return nc


def _split16(x):
    hi = x.astype(np.float16)
    lo = (x - hi.astype(np.float32)).astype(np.float16)
    return hi, lo


def _split16_3(x):
    h = x.astype(np.float16)
    r = x - h.astype(np.float32)
    m = r.astype(np.float16)
    l = (r - m.astype(np.float32)).astype(np.float16)
    return h, m, l


def kernel(t, y, masses, G):
    global LAST_RUN
    from concourse.bass_utils import run_bass_kernel_spmd

    y = np.asarray(y, np.float32).reshape(N, 3)
    m = np.asarray(masses, np.float32).reshape(N)
    g = np.float32(np.asarray(G).reshape(()))

    d2 = (y * y).sum(1, dtype=np.float32)
    ones = np.ones(N, np.float32)
    a = np.stack([y[:, 0], y[:, 1], y[:, 2], d2, ones])  # [5, N] fp32
    b = np.stack([-2 * y[:, 0], -2 * y[:, 1], -2 * y[:, 2], ones, d2 + EPS])
    ah, am, al = _split16_3(a)
    bh, bm, bl = _split16_3(b)
    # (ah+am+al).(bh+bm+bl) expanded, keeping pairs whose product can reach
    # ~2^-22 of z: (h,h) (h,m) (m,h) (h,l) (l,h) (m,m); dropped terms < 2^-33.
    aT30 = np.concatenate([ah, ah, am, ah, al, am], axis=0)  # [30, N]
    bT30_full = np.concatenate([bh, bm, bh, bl, bh, bm], axis=0)  # [30, N]
    # pack pairs of i-tiles into row strips at partitions 0-29 / 32-61
    a3 = aT30.reshape(KF, ITILES // 2, 2, P)  # [30, 32, 2, 128]
    aTP = np.zeros((P, (ITILES // 2) * P), np.float16)
    aTP_v = aTP.reshape(P, ITILES // 2, P)
    aTP_v[0:KF] = a3[:, :, 0, :]
    aTP_v[32 : 32 + KF] = a3[:, :, 1, :]
    yp = np.concatenate([y, ones[:, None]], axis=1)  # [N, 4] fp32
    yph, ypl = _split16(yp)
    ypc = np.ascontiguousarray(np.concatenate([yph, ypl], axis=1))  # [N, 8]

    if "nc" not in _cache:
        _cache["nc"] = _build()
    nc = _cache["nc"]

    in_maps = []
    for c in range(NCORES):
        bT_loc = bT30_full[:, c * JL : (c + 1) * JL]
        bTP = np.zeros((P, JL), np.float16)
        bTP[0:KF] = bT_loc
        bTP[32 : 32 + KF] = bT_loc
        in_maps.append({"aTP": aTP, "bTP": bTP, "ypc": ypc})
    LAST_RUN = run_bass_kernel_spmd(nc, in_maps, core_ids=list(range(NCORES)))
    S8 = np.concatenate([r["S"] for r in LAST_RUN.results], axis=1)  # [8, N]
    S = S8[0:4] + S8[4:8]
    acc = (np.float32(ICD_SCALE) * g * m)[:, None] * (S[0:3].T - y * S[3][:, None])
    return acc.astype(np.float32)


# revision 26
# speedup vs baseline: 1.2905x; 1.0349x over previous
"""Direct N-body gravitational acceleration on 8 Trainium2 NeuronCores.

Sharding: target-particle axis j split across the 8 cores (1024 targets
per core); every core holds the full (N,3) source positions.

Math (reference):
    z[i,j]   = |y_i - y_j|^2 + eps
    icd[i,j] = z^{-3/2}
    acc[j]   = G*m_j * (sum_i icd[i,j]*y_i  -  y_j * sum_i icd[i,j])

Per-core device pipeline (everything O(N^2) stays on-chip):
    mm1 (PE): z[i,j] = a_i . b_j with a_i=(y_i, d2_i, 1),
        b_j=(-2*y_j, 1, d2_j+eps).  To avoid the 4x-cost fp32 matmul
        path each fp32 feature is 3-way fp16 split and the product
        expanded into 6 cross terms -> one K=30 fp16 matmul with
        fp32-grade accuracy.
    ACT: t = Abs_reciprocal_sqrt(z) = z^{-1/2}  (HW-measured 4.4e-5
        max rel err), one pass.
    DVE (custom fused op): icd/16 = min(t, 100)^3 / 16 written as fp16.
        The clamp bounds fp16 at 62500 (z rounding can dip below eps on
        the diagonal; true off-diagonal pairs sit well above the clamp).
    mm2 (PE, fp16, K=128 per i-tile, PSUM-accumulated over 64 i-tiles):
        S[c,j] += sum_i yp[i,c]*icd[i,j], with yp = [y | 1] split into
        fp16 hi+lo halves packed as one [128, 8] weight (the y_i - y_j
        cancellation must survive quantization) -> one matmul per
        (i-tile, j-half).
Host does the O(N) prep (feature splits) and the O(N) affine combine
    acc[j] = 16*G*m_j*(S[0:3,j] - y_j*S[3,j]) with S = S_hi + S_lo.
"""

import numpy as np

N = 8192
NCORES = 8
JL = N // NCORES  # 1024 local targets per core
P = 128
ITILES = N // P  # 64
EPS = np.float32(0.01 * 0.01)
ICD_SCALE = 16.0  # icd stored as icd/16 in fp16
T_CLAMP = 100.0  # = EPS**-0.5; min(t, clamp)^3/16 = 62500 < fp16 max
KF = 30  # feature rows after 3-way fp16 split (6 kept cross-product pairs)

_cache: dict = {}
LAST_RUN = None  # BassKernelResults of the most recent launch (for test.py)


def _register_cube_op():
    """Register a fused clamp+cube+scale custom DVE op (the documented
    dve_ops extension point, applied at runtime since the repo is read-only):
    out = min(in0, s0)^3 * s1."""
    import concourse.dve_ops as dve_ops
    from concourse.dve_spec import Spec, Src0, C0, C1, lower, minn, sq
    from concourse.dve_uop import DveOpSpec

    name = "CUBE_CLAMP_SCALE_NB"
    for op in dve_ops.OPS:
        if op.name == name:
            return op

    m = minn(Src0, C0)
    spec = Spec(
        body=sq(m) * m * C1,
        reference=lambda in0, in1, s0, s1, imm2: (
            np.minimum(in0.astype(np.float32), s0) ** 3 * s1
        ),
    )
    row = dve_ops._CUSTOM_DVE_ROW_BASE + len(dve_ops.OPS)
    shas = {}
    for ver in ("v3", "v4"):
        try:
            uops = lower(spec, ver=ver)
        except Exception:
            continue
        shas[ver] = DveOpSpec(name=name, opcode=row, uops=uops, rd1_en=False).sha(ver)
    op = dve_ops.DveOp(name, spec, subdim=False, uops_sha=shas)
    dve_ops.OPS.append(op)
    dve_ops.CUSTOM_DVE_SPECS[name] = spec
    dve_ops._SUB_OPCODE_FOR_NAME[name] = row
    return op


def _enable_ldw_opt():
    """The stock compile driver passes --enable-ldw-opt=false, which forces a
    serializing LDWEIGHTS before every matmul.  Enable the walrus LDW
    optimization (dedup/background-buffer loads) for this kernel's compiles."""
    import concourse.bass_utils as bu

    if getattr(bu, "_ldw_opt_patched", False):
        return
    orig = bu.bir_verify_and_optimise

    def patched(tmpdir, inp="bir.json", outp="file.neff", arch=None, *, dve_root=None):
        import unittest.mock as mock

        real_run = bu.run_command

        def run_patched(argv, **kw):
            argv = [
                a.replace("--enable-ldw-opt=false", "--enable-ldw-opt=true")
                for a in argv
            ]
            return real_run(argv, **kw)

        with mock.patch.object(bu, "run_command", run_patched):
            return orig(tmpdir, inp, outp, arch, dve_root=dve_root)

    bu.bir_verify_and_optimise = patched
    bu._ldw_opt_patched = True


def _build():
    import concourse.bacc as bacc
    import concourse.mybir as mybir
    import concourse.tile as tile

    f32 = mybir.dt.float32
    f16 = mybir.dt.float16
    AF = mybir.ActivationFunctionType

    cube_op = _register_cube_op()

    nc = bacc.Bacc("TRN2", target_bir_lowering=False, debug=False)
    # aTP packs the K=30 feature panels of two consecutive i-tiles into row
    # strips at partitions 0-29 and 32-61 so the two mm1 matmuls of a pair
    # run concurrently in different PE row groups (tile_position row tiling).
    aTP = nc.dram_tensor("aTP", [P, (N // 2)], f16, kind="ExternalInput")
    bTP = nc.dram_tensor("bTP", [P, JL], f16, kind="ExternalInput")
    ypc = nc.dram_tensor("ypc", [N, 8], f16, kind="ExternalInput")
    S = nc.dram_tensor("S", [8, JL], f32, kind="ExternalOutput")

    with tile.TileContext(nc) as tc:
        with (
            tc.tile_pool(name="const", bufs=1) as cpool,
            tc.tile_pool(name="tp", bufs=4) as tpool,
            tc.tile_pool(name="icdp", bufs=4) as icdpool,
            tc.tile_pool(name="ps1", bufs=2, space="PSUM") as ps1pool,
            tc.tile_pool(name="scr", bufs=1, space="PSUM") as scrpool,
            tc.tile_pool(name="ps2", bufs=1, space="PSUM") as ps2pool,
            tc.tile_pool(name="outp", bufs=1) as opool,
        ):
            aTP_sb = cpool.tile([P, N // 2], f16)
            nc.sync.dma_start(aTP_sb[:], aTP[:])
            bTP_sb = cpool.tile([P, JL], f16)
            nc.sync.dma_start(bTP_sb[:], bTP[:])
            ypc_sb = cpool.tile([P, ITILES, 8], f16)
            nc.sync.dma_start(ypc_sb[:], ypc.rearrange("(t p) c -> p t c", p=P))

            ps2a = ps2pool.tile([8, 512], f32)
            ps2b = ps2pool.tile([8, 512], f32)

            # PE warm-up: ~10us of dense dependency-free matmuls on a zeroed
            # tile trips the HAM clock gate to 8/8 (2.4 GHz) before the main
            # loop; otherwise the whole kernel can run at the cold 1.2 GHz.
            warm_in = cpool.tile([P, 512], f16)
            nc.vector.memset(warm_in[:], 0.0)
            scr = scrpool.tile([P, 512], f32)
            for w in range(16):
                nc.tensor.matmul(
                    scr[:], warm_in[:, 0:128], warm_in[:], start=True, stop=True
                )

            def emit_dummy_mm():
                # Dependency-free filler matmul.  The HAM clock gate
                # re-throttles the PE to 1.2 GHz whenever its busy fraction
                # drops; real work alone leaves the warm-clock PE ~70% busy,
                # which makes the throttle bi-stable at cold.  Padding the PE
                # queue keeps the duty cycle ~100% at 2.4 GHz, which is a
                # large net win.
                nc.tensor.matmul(
                    scr[:], warm_in[:, 0:128], warm_in[:], start=True, stop=True
                )

            NG = ITILES // 2  # 32 groups of 2 i-tiles

            def emit_mm1_pair(g):
                # two concurrent row-tiled matmuls (strips at partitions 0/32)
                # compute z for i-tiles 2g and 2g+1
                tiles = []
                for r in (0, 1):
                    ps1 = ps1pool.tile([P, JL], f32, tag="ps1t")
                    lhs = aTP_sb[32 * r : 32 * r + KF, g * P : (g + 1) * P]
                    for jh in (0, 1):
                        nc.tensor.matmul(
                            ps1[:, jh * 512 : (jh + 1) * 512],
                            lhs,
                            bTP_sb[32 * r : 32 * r + KF, jh * 512 : (jh + 1) * 512],
                            start=True,
                            stop=True,
                            tile_position=(32 * r, 0),
                        )
                    tiles.append(ps1)
                return tiles

            # software pipeline: mm1 for group g+1 is queued before mm2 of
            # group g so the PE always has dependency-free work in flight
            cur = emit_mm1_pair(0)
            for g in range(NG):
                icds = []
                for r in (0, 1):
                    t = tpool.tile([P, JL], f32)
                    nc.scalar.activation(t[:], cur[r][:], AF.Abs_reciprocal_sqrt)
                    icd = icdpool.tile([P, JL], f16)
                    nc.vector._custom_dve(
                        cube_op, out=icd[:], in0=t[:], s0=T_CLAMP, s1=1.0 / ICD_SCALE
                    )
                    icds.append(icd)
                if g + 1 < NG:
                    cur = emit_mm1_pair(g + 1)
                for r in (0, 1):
                    it = 2 * g + r
                    first, last = it == 0, it == ITILES - 1
                    nc.tensor.matmul(
                        ps2a[:],
                        ypc_sb[:, it, :],
                        icds[r][:, 0:512],
                        start=first,
                        stop=last,
                    )
                    nc.tensor.matmul(
                        ps2b[:],
                        ypc_sb[:, it, :],
                        icds[r][:, 512:1024],
                        start=first,
                        stop=last,
                    )
                    emit_dummy_mm()
            S_sb = opool.tile([8, JL], f32)
            nc.vector.tensor_copy(S_sb[:, 0:512], ps2a[:])
            nc.vector.tensor_copy(S_sb[:, 512:1024], ps2b[:])
            nc.sync.dma_start(S[:], S_sb[:])
    nc.compile()
    return nc


def _split16(x):
    hi = x.astype(np.float16)
    lo = (x - hi.astype(np.float32)).astype(np.float16)
    return hi, lo


def _split16_3(x):
    h = x.astype(np.float16)
    r = x - h.astype(np.float32)
    m = r.astype(np.float16)
    l = (r - m.astype(np.float32)).astype(np.float16)
    return h, m, l


def kernel(t, y, masses, G):
    global LAST_RUN
    from concourse.bass_utils import run_bass_kernel_spmd

    y = np.asarray(y, np.float32).reshape(N, 3)
    m = np.asarray(masses, np.float32).reshape(N)
    g = np.float32(np.asarray(G).reshape(()))

    d2 = (y * y).sum(1, dtype=np.float32)
    ones = np.ones(N, np.float32)
    a = np.stack([y[:, 0], y[:, 1], y[:, 2], d2, ones])  # [5, N] fp32
    b = np.stack([-2 * y[:, 0], -2 * y[:, 1], -2 * y[:, 2], ones, d2 + EPS])
    ah, am, al = _split16_3(a)
    bh, bm, bl = _split16_3(b)
    # (ah+am+al).(bh+bm+bl) expanded, keeping pairs whose product can reach
    # ~2^-22 of z: (h,h) (h,m) (m,h) (h,l) (l,h) (m,m); dropped terms < 2^-33.
    aT30 = np.concatenate([ah, ah, am, ah, al, am], axis=0)  # [30, N]
    bT30_full = np.concatenate([bh, bm, bh, bl, bh, bm], axis=0)  # [30, N]
    # pack pairs of i-tiles into row strips at partitions 0-29 / 32-61
    a3 = aT30.reshape(KF, ITILES // 2, 2, P)  # [30, 32, 2, 128]
    aTP = np.zeros((P, (ITILES // 2) * P), np.float16)
    aTP_v = aTP.reshape(P, ITILES // 2, P)
    aTP_v[0:KF] = a3[:, :, 0, :]
    aTP_v[32 : 32 + KF] = a3[:, :, 1, :]
    yp = np.concatenate([y, ones[:, None]], axis=1)  # [N, 4] fp32
    yph, ypl = _split16(yp)
    ypc = np.ascontiguousarray(np.concatenate([yph, ypl], axis=1))  # [N, 8]

    if "nc" not in _cache:
        _cache["nc"] = _build()
    nc = _cache["nc"]

    in_maps = []
    for c in range(NCORES):
        bT_loc = bT30_full[:, c * JL : (c + 1) * JL]
        bTP = np.zeros((P, JL), np.float16)
        bTP[0:KF] = bT_loc
        bTP[32 : 32 + KF] = bT_loc
        in_maps.append({"aTP": aTP, "bTP": bTP, "ypc": ypc})
    LAST_RUN = run_bass_kernel_spmd(nc, in_maps, core_ids=list(range(NCORES)))
    S8 = np.concatenate([r["S"] for r in LAST_RUN.results], axis=1)  # [8, N]
    S = S8[0:4] + S8[4:8]
    acc = (np.float32(ICD_SCALE) * g * m)[:, None] * (S[0:3].T - y * S[3][:, None])
    return acc.astype(np.float32)


# revision 27
# speedup vs baseline: 1.3935x; 1.0798x over previous
"""Direct N-body gravitational acceleration on 8 Trainium2 NeuronCores.

Sharding: target-particle axis j split across the 8 cores (1024 targets
per core); every core holds the full (N,3) source positions.

Math (reference):
    z[i,j]   = |y_i - y_j|^2 + eps
    icd[i,j] = z^{-3/2}
    acc[j]   = G*m_j * (sum_i icd[i,j]*y_i  -  y_j * sum_i icd[i,j])

Per-core device pipeline (everything O(N^2) stays on-chip):
    mm1 (PE): z[i,j] = a_i . b_j with a_i=(y_i, d2_i, 1),
        b_j=(-2*y_j, 1, d2_j+eps).  To avoid the 4x-cost fp32 matmul
        path each fp32 feature is 3-way fp16 split and the product
        expanded into 6 cross terms -> one K=30 fp16 matmul with
        fp32-grade accuracy.
    ACT: t = Abs_reciprocal_sqrt(z) = z^{-1/2}  (HW-measured 4.4e-5
        max rel err), one pass.
    DVE (custom fused op): icd/16 = min(t, 100)^3 / 16 written as fp16.
        The clamp bounds fp16 at 62500 (z rounding can dip below eps on
        the diagonal; true off-diagonal pairs sit well above the clamp).
    mm2 (PE, fp16, K=128 per i-tile, PSUM-accumulated over 64 i-tiles):
        S[c,j] += sum_i yp[i,c]*icd[i,j], with yp = [y | 1] split into
        fp16 hi+lo halves packed as one [128, 8] weight (the y_i - y_j
        cancellation must survive quantization) -> one matmul per
        (i-tile, j-half).
Host does the O(N) prep (feature splits) and the O(N) affine combine
    acc[j] = 16*G*m_j*(S[0:3,j] - y_j*S[3,j]) with S = S_hi + S_lo.
"""

import numpy as np

N = 8192
NCORES = 8
JL = N // NCORES  # 1024 local targets per core
P = 128
ITILES = N // P  # 64
EPS = np.float32(0.01 * 0.01)
ICD_SCALE = 16.0  # icd stored as icd/16 in fp16
T_CLAMP = 100.0  # = EPS**-0.5; min(t, clamp)^3/16 = 62500 < fp16 max
KF = 30  # feature rows after 3-way fp16 split (6 kept cross-product pairs)

_cache: dict = {}
LAST_RUN = None  # BassKernelResults of the most recent launch (for test.py)


def _register_cube_op():
    """Register a fused clamp+cube+scale custom DVE op (the documented
    dve_ops extension point, applied at runtime since the repo is read-only):
    out = min(in0, s0)^3 * s1."""
    import concourse.dve_ops as dve_ops
    from concourse.dve_spec import Spec, Src0, C0, C1, lower, minn, sq
    from concourse.dve_uop import DveOpSpec

    name = "CUBE_CLAMP_SCALE_NB"
    for op in dve_ops.OPS:
        if op.name == name:
            return op

    m = minn(Src0, C0)
    spec = Spec(
        body=sq(m) * m * C1,
        reference=lambda in0, in1, s0, s1, imm2: (
            np.minimum(in0.astype(np.float32), s0) ** 3 * s1
        ),
    )
    row = dve_ops._CUSTOM_DVE_ROW_BASE + len(dve_ops.OPS)
    shas = {}
    for ver in ("v3", "v4"):
        try:
            uops = lower(spec, ver=ver)
        except Exception:
            continue
        shas[ver] = DveOpSpec(name=name, opcode=row, uops=uops, rd1_en=False).sha(ver)
    op = dve_ops.DveOp(name, spec, subdim=False, uops_sha=shas)
    dve_ops.OPS.append(op)
    dve_ops.CUSTOM_DVE_SPECS[name] = spec
    dve_ops._SUB_OPCODE_FOR_NAME[name] = row
    return op


def _enable_ldw_opt():
    """The stock compile driver passes --enable-ldw-opt=false, which forces a
    serializing LDWEIGHTS before every matmul.  Enable the walrus LDW
    optimization (dedup/background-buffer loads) for this kernel's compiles."""
    import concourse.bass_utils as bu

    if getattr(bu, "_ldw_opt_patched", False):
        return
    orig = bu.bir_verify_and_optimise

    def patched(tmpdir, inp="bir.json", outp="file.neff", arch=None, *, dve_root=None):
        import unittest.mock as mock

        real_run = bu.run_command

        def run_patched(argv, **kw):
            argv = [
                a.replace("--enable-ldw-opt=false", "--enable-ldw-opt=true")
                for a in argv
            ]
            return real_run(argv, **kw)

        with mock.patch.object(bu, "run_command", run_patched):
            return orig(tmpdir, inp, outp, arch, dve_root=dve_root)

    bu.bir_verify_and_optimise = patched
    bu._ldw_opt_patched = True


def _build():
    import concourse.bacc as bacc
    import concourse.mybir as mybir
    import concourse.tile as tile

    f32 = mybir.dt.float32
    f16 = mybir.dt.float16
    AF = mybir.ActivationFunctionType

    cube_op = _register_cube_op()

    nc = bacc.Bacc("TRN2", target_bir_lowering=False, debug=False)
    # aTP packs the K=30 feature panels of two consecutive i-tiles into row
    # strips at partitions 0-29 and 32-61 so the two mm1 matmuls of a pair
    # run concurrently in different PE row groups (tile_position row tiling).
    aTP = nc.dram_tensor("aTP", [P, (N // 2)], f16, kind="ExternalInput")
    bTP = nc.dram_tensor("bTP", [P, JL], f16, kind="ExternalInput")
    ypc = nc.dram_tensor("ypc", [N, 8], f16, kind="ExternalInput")
    S = nc.dram_tensor("S", [8, JL], f32, kind="ExternalOutput")

    with tile.TileContext(nc) as tc:
        with (
            tc.tile_pool(name="const", bufs=1) as cpool,
            tc.tile_pool(name="tp", bufs=4) as tpool,
            tc.tile_pool(name="icdp", bufs=4) as icdpool,
            tc.tile_pool(name="ps1", bufs=2, space="PSUM") as ps1pool,
            tc.tile_pool(name="scr", bufs=1, space="PSUM") as scrpool,
            tc.tile_pool(name="ps2", bufs=1, space="PSUM") as ps2pool,
            tc.tile_pool(name="outp", bufs=1) as opool,
        ):
            aTP_sb = cpool.tile([P, N // 2], f16)
            nc.sync.dma_start(aTP_sb[:], aTP[:])
            bTP_sb = cpool.tile([P, JL], f16)
            nc.sync.dma_start(bTP_sb[:], bTP[:])
            ypc_sb = cpool.tile([P, ITILES, 8], f16)
            nc.sync.dma_start(ypc_sb[:], ypc.rearrange("(t p) c -> p t c", p=P))

            ps2a = ps2pool.tile([8, 512], f32)
            ps2b = ps2pool.tile([8, 512], f32)

            # PE warm-up: ~10us of dense dependency-free matmuls on a zeroed
            # tile trips the HAM clock gate to 8/8 (2.4 GHz) before the main
            # loop; otherwise the whole kernel can run at the cold 1.2 GHz.
            warm_in = cpool.tile([P, 512], f16)
            nc.vector.memset(warm_in[:], 0.0)
            scr = scrpool.tile([P, 512], f32)
            for w in range(16):
                nc.tensor.matmul(
                    scr[:], warm_in[:, 0:128], warm_in[:], start=True, stop=True
                )

            def emit_dummy_mm():
                # Dependency-free filler matmul.  The HAM clock gate
                # re-throttles the PE to 1.2 GHz whenever its busy fraction
                # drops; real work alone leaves the warm-clock PE ~70% busy,
                # which makes the throttle bi-stable at cold.  Padding the PE
                # queue keeps the duty cycle ~100% at 2.4 GHz, which is a
                # large net win.
                nc.tensor.matmul(
                    scr[:], warm_in[:, 0:128], warm_in[:], start=True, stop=True
                )

            NG = ITILES // 2  # 32 groups of 2 i-tiles

            def emit_mm1_pair(g):
                # two concurrent row-tiled matmuls (strips at partitions 0/32)
                # compute z for i-tiles 2g and 2g+1
                tiles = []
                for r in (0, 1):
                    ps1 = ps1pool.tile([P, JL], f32, tag="ps1t")
                    lhs = aTP_sb[32 * r : 32 * r + KF, g * P : (g + 1) * P]
                    for jh in (0, 1):
                        nc.tensor.matmul(
                            ps1[:, jh * 512 : (jh + 1) * 512],
                            lhs,
                            bTP_sb[32 * r : 32 * r + KF, jh * 512 : (jh + 1) * 512],
                            start=True,
                            stop=True,
                            tile_position=(32 * r, 0),
                        )
                    tiles.append(ps1)
                return tiles

            # software pipeline: mm1 for group g+1 is queued before mm2 of
            # group g so the PE always has dependency-free work in flight
            cur = emit_mm1_pair(0)
            for g in range(NG):
                icds = []
                for r in (0, 1):
                    t = tpool.tile([P, JL], f32)
                    nc.scalar.activation(t[:], cur[r][:], AF.Abs_reciprocal_sqrt)
                    icd = icdpool.tile([P, JL], f16)
                    nc.vector._custom_dve(
                        cube_op, out=icd[:], in0=t[:], s0=T_CLAMP, s1=1.0 / ICD_SCALE
                    )
                    icds.append(icd)
                if g + 1 < NG:
                    cur = emit_mm1_pair(g + 1)
                for r in (0, 1):
                    it = 2 * g + r
                    first, last = it == 0, it == ITILES - 1
                    nc.tensor.matmul(
                        ps2a[:],
                        ypc_sb[:, it, :],
                        icds[r][:, 0:512],
                        start=first,
                        stop=last,
                    )
                    nc.tensor.matmul(
                        ps2b[:],
                        ypc_sb[:, it, :],
                        icds[r][:, 512:1024],
                        start=first,
                        stop=last,
                    )
                    if r == 0:
                        emit_dummy_mm()
            S_sb = opool.tile([8, JL], f32)
            nc.vector.tensor_copy(S_sb[:, 0:512], ps2a[:])
            nc.vector.tensor_copy(S_sb[:, 512:1024], ps2b[:])
            nc.sync.dma_start(S[:], S_sb[:])
    nc.compile()
    return nc


def _split16(x):
    hi = x.astype(np.float16)
    lo = (x - hi.astype(np.float32)).astype(np.float16)
    return hi, lo


def _split16_3(x):
    h = x.astype(np.float16)
    r = x - h.astype(np.float32)
    m = r.astype(np.float16)
    l = (r - m.astype(np.float32)).astype(np.float16)
    return h, m, l


def kernel(t, y, masses, G):
    global LAST_RUN
    from concourse.bass_utils import run_bass_kernel_spmd

    y = np.asarray(y, np.float32).reshape(N, 3)
    m = np.asarray(masses, np.float32).reshape(N)
    g = np.float32(np.asarray(G).reshape(()))

    d2 = (y * y).sum(1, dtype=np.float32)
    ones = np.ones(N, np.float32)
    a = np.stack([y[:, 0], y[:, 1], y[:, 2], d2, ones])  # [5, N] fp32
    b = np.stack([-2 * y[:, 0], -2 * y[:, 1], -2 * y[:, 2], ones, d2 + EPS])
    ah, am, al = _split16_3(a)
    bh, bm, bl = _split16_3(b)
    # (ah+am+al).(bh+bm+bl) expanded, keeping pairs whose product can reach
    # ~2^-22 of z: (h,h) (h,m) (m,h) (h,l) (l,h) (m,m); dropped terms < 2^-33.
    aT30 = np.concatenate([ah, ah, am, ah, al, am], axis=0)  # [30, N]
    bT30_full = np.concatenate([bh, bm, bh, bl, bh, bm], axis=0)  # [30, N]
    # pack pairs of i-tiles into row strips at partitions 0-29 / 32-61
    a3 = aT30.reshape(KF, ITILES // 2, 2, P)  # [30, 32, 2, 128]
    aTP = np.zeros((P, (ITILES // 2) * P), np.float16)
    aTP_v = aTP.reshape(P, ITILES // 2, P)
    aTP_v[0:KF] = a3[:, :, 0, :]
    aTP_v[32 : 32 + KF] = a3[:, :, 1, :]
    yp = np.concatenate([y, ones[:, None]], axis=1)  # [N, 4] fp32
    yph, ypl = _split16(yp)
    ypc = np.ascontiguousarray(np.concatenate([yph, ypl], axis=1))  # [N, 8]

    if "nc" not in _cache:
        _cache["nc"] = _build()
    nc = _cache["nc"]

    in_maps = []
    for c in range(NCORES):
        bT_loc = bT30_full[:, c * JL : (c + 1) * JL]
        bTP = np.zeros((P, JL), np.float16)
        bTP[0:KF] = bT_loc
        bTP[32 : 32 + KF] = bT_loc
        in_maps.append({"aTP": aTP, "bTP": bTP, "ypc": ypc})
    LAST_RUN = run_bass_kernel_spmd(nc, in_maps, core_ids=list(range(NCORES)))
    S8 = np.concatenate([r["S"] for r in LAST_RUN.results], axis=1)  # [8, N]
    S = S8[0:4] + S8[4:8]
    acc = (np.float32(ICD_SCALE) * g * m)[:, None] * (S[0:3].T - y * S[3][:, None])
    return acc.astype(np.float32)
